# revision 39
# baseline (speedup 1.0000x reference)
"""Causal self-attention with RoPE on 8 TRN2 NeuronCores.

Head-parallel tensor parallelism: core i owns heads 2i, 2i+1. Each core
computes its slice of the qkv projection (bf16 inputs, f32 accumulate),
RoPE via a permutation matmul + DVE muls, then per-head causal
attention entirely in SBUF (bf16 q/k/v, f32 PSUM, exp on the Act
engine with the diagonal blocks column-restricted to the unmasked
range). The softmax-normalized per-head outputs are exchanged with a
per-batch AllToAll (8x less traffic than a ReduceScatter of partial
projections) so every core holds all 1024 attention channels for its
512-row t-shard; the output projection then runs locally over the full
contraction with no cross-core reduction. outproj(batch 0) is emitted
after the batch-1 AllToAll trigger so it fills the collective's wait
window.

PSUM: 1 bank scratch (rope/transpose/den-broadcast), 3 banks for the
S tiles (deep S->exp->AV pipeline) shared with outproj, 4 banks for
the AV accumulators shared with the qkv projection.

Erratum rules respected: no f32r transpose (V transposed in plain f32),
no mixed base partitions inside one PSUM accumulation group, no
bf16/f32r operand mixing in matmuls, no PSUMxPSUM tensor_tensor, no
PSUM or partition-offset inputs to the custom DVE reciprocal ops.
"""

import numpy as np
import ml_dtypes

import concourse.bass as bass
import concourse.mybir as mybir
import concourse.tile as tile
from concourse import bacc
from concourse.bass_utils import run_bass_kernel_spmd
from concourse.dve_ops import RECIPROCAL_APPROX_NR

F32 = mybir.dt.float32
F32R = mybir.dt.float32r
BF16 = mybir.dt.bfloat16

B, T, C = 2, 2048, 1024
H, HD = 16, 64
NC = 8
HL = H // NC          # heads per core = 2
BT = B * T            # 4096
FQKV = 3 * HL * HD    # 384 rows of w_attn per core
TSH = BT // NC        # 512 output rows per core (256 per batch)
NCH = BT // 512       # 8 column chunks of the [*, BT] activations
ROPE_BASE = 10000.0
BF = ml_dtypes.bfloat16


def build():
    nc = bacc.Bacc(None, target_bir_lowering=False)

    xT_d = nc.dram_tensor("xT", [C, BT], BF16, kind="ExternalInput")
    wq_d = nc.dram_tensor("wqkvT", [C, FQKV], BF16, kind="ExternalInput")
    wp_d = nc.dram_tensor("wpT", [C, C], BF16, kind="ExternalInput")
    cos_d = nc.dram_tensor("cosT", [128, BT], BF16, kind="ExternalInput")
    sin_d = nc.dram_tensor("sinT", [128, BT], BF16, kind="ExternalInput")
    perm_d = nc.dram_tensor("permT", [128, 128], BF16, kind="ExternalInput")
    mask_d = nc.dram_tensor("masks", [4, 128, 512], BF16, kind="ExternalInput")
    id_d = nc.dram_tensor("ident", [128, 128], F32, kind="ExternalInput")
    out_d = nc.dram_tensor("out", [TSH, C], F32, kind="ExternalOutput")

    # AllToAll exchange: block j of a2a_in = my 128 attention channels for
    # core j's 256 t-rows of batch b; block j of a2a_out = core j's 128
    # channels for MY 256 t-rows.
    a2a_in_m = nc.dram_tensor("a2ainm", [1024, 512], BF16)
    a2a_out_m = nc.dram_tensor("a2aoutm", [1024, 512], BF16)
    a2a_in = {b_: a2a_in_m[:, 256 * b_:256 * (b_ + 1)] for b_ in range(2)}
    a2a_out = {b_: a2a_out_m[:, 256 * b_:256 * (b_ + 1)] for b_ in range(2)}

    with tile.TileContext(nc) as tc:
        with (
            tc.tile_pool(name="persist", bufs=1) as pp,
            tc.tile_pool(name="work", bufs=2) as wk,
            tc.tile_pool(name="pts", bufs=12) as ptp,
            tc.tile_pool(name="psA", bufs=1, space="PSUM") as psA,
            tc.tile_pool(name="psS", bufs=3, space="PSUM") as psS,
            tc.tile_pool(name="psV", bufs=4, space="PSUM") as psV,
        ):
            # ---- constants / weights (persist) ----
            wq_sb = []
            for c in range(8):
                t = pp.tile([128, FQKV], BF16, name=f"wq{c}", tag=f"wq{c}")
                nc.sync.dma_start(t[:], wq_d[c * 128:(c + 1) * 128, :])
                wq_sb.append(t)
            wp_sb = []
            for c in range(8):
                t = pp.tile([128, C], BF16, name=f"wp{c}", tag=f"wp{c}")
                nc.gpsimd.dma_start(t[:], wp_d[c * 128:(c + 1) * 128, :])
                wp_sb.append(t)
            perm_sb = pp.tile([128, 128], BF16, name="perm_sb", tag="perm_sb")
            nc.gpsimd.dma_start(perm_sb[:], perm_d[:])
            id_sb = pp.tile([128, 128], F32, name="id_sb", tag="id_sb")
            nc.gpsimd.dma_start(id_sb[:], id_d[:])
            mask_sb = []
            for m in range(4):
                t = pp.tile([128, 512], BF16, name=f"mask{m}", tag=f"mask{m}")
                nc.gpsimd.dma_start(t[:], mask_d[m])
                mask_sb.append(t)
            ones_f = pp.tile([128, 1], F32, name="ones_f", tag="ones_f")
            nc.vector.memset(ones_f[:], 1.0)
            ones_c = pp.tile([128, 1], BF16, name="ones_c", tag="ones_c")
            nc.vector.tensor_copy(ones_c[:], ones_f[:])
            ones_r = pp.tile([1, HD], F32R, name="ones_r", tag="ones_r")
            nc.vector.tensor_copy(ones_r[:],
                                  ones_f[0:1, 0:1].broadcast_to((1, HD)))


            # chunked activations: 8 chunks of [128, 512] each
            qtc = [pp.tile([128, 512], BF16, name=f"qtc{i}", tag=f"qtc{i}")
                   for i in range(NCH)]
            ktc = [pp.tile([128, 512], BF16, name=f"ktc{i}", tag=f"ktc{i}")
                   for i in range(NCH)]
            vtc = [pp.tile([128, 512], F32, name=f"vtc{i}", tag=f"vtc{i}")
                   for i in range(NCH)]
            fdst = [qtc, ktc, vtc]
            v_sb = [None] * 32

            # ---- phase 1: qkvT = wqkvT.T @ xT, [f, t] layout ----
            def qkv_quarter(th):
                xt_sb = []
                for c in range(8):
                    t = pp.tile([128, 1024], BF16, name=f"xt{th}{c}",
                                tag=f"xt{c}")
                    nc.sync.dma_start(t[:], xT_d[c * 128:(c + 1) * 128,
                                                 th * 1024:(th + 1) * 1024])
                    xt_sb.append(t)
                for f in range(3):
                    pq = [psV.tile([128, 512], F32, name=f"pq{th}{f}{tq}",
                                   tag="ps_av") for tq in range(2)]
                    for c in range(8):
                        for tq in range(2):
                            nc.tensor.matmul(
                                pq[tq][:],
                                wq_sb[c][:, f * 128:(f + 1) * 128],
                                xt_sb[c][:, tq * 512:(tq + 1) * 512],
                                start=(c == 0), stop=(c == 7),
                            )
                    for tq in range(2):
                        nc.vector.tensor_copy(fdst[f][th * 2 + tq][:],
                                              pq[tq][:])

            # ---- phase 2: RoPE on q, k chunks (in place) ----
            def rope_chunk(ch):
                cosc = wk.tile([128, 512], BF16, name=f"cosc{ch}", tag="cosc")
                nc.gpsimd.dma_start(cosc[:], cos_d[:, ch * 512:(ch + 1) * 512])
                sinc = wk.tile([128, 512], BF16, name=f"sinc{ch}", tag="sinc")
                nc.gpsimd.dma_start(sinc[:], sin_d[:, ch * 512:(ch + 1) * 512])
                for which, tcl in (("q", qtc), ("k", ktc)):
                    src = tcl[ch]
                    pr = psA.tile([128, 512], F32, name=f"pr{which}{ch}",
                                  tag="ps_a")
                    nc.tensor.matmul(pr[:], perm_sb[:], src[:],
                                     start=True, stop=True)
                    rot = wk.tile([128, 512], BF16, name=f"rot{which}{ch}",
                                  tag="rot")
                    nc.vector.tensor_mul(rot[:], pr[:], sinc[:])
                    nc.vector.tensor_mul(src[:], src[:], cosc[:])
                    nc.vector.tensor_add(src[:], src[:], rot[:])

            # ---- phase 3: V blocks [t, d] with ones columns ----
            def v_block(kb):
                pv = psA.tile([128, 128], F32, name=f"pv{kb}", tag="ps_a")
                nc.tensor.transpose(
                    pv[:], vtc[kb // 4][:, (kb % 4) * 128:(kb % 4 + 1) * 128],
                    id_sb[:])
                v = pp.tile([128, 2 * (HD + 1)], BF16, name=f"v{kb}",
                            tag=f"v{kb}")
                nc.vector.tensor_copy(v[:, 0:HD], pv[:, 0:HD])
                nc.vector.tensor_copy(v[:, HD + 1:2 * HD + 1],
                                      pv[:, HD:2 * HD])
                nc.vector.tensor_copy(v[:, HD:HD + 1], ones_c[:])
                nc.vector.tensor_copy(v[:, 2 * HD + 1:2 * HD + 2], ones_c[:])
                v_sb[kb] = v

            # ---- phase 4: attention per (batch, head), kb-outer ----
            def normalize(b, h, qc, avq):
                """softmax-normalize avq rows 0:64 by the ones-row 64 and
                stage the result into the AllToAll input buffer."""
                den = wk.tile([1, 512], F32, name=f"den{b}{h}{qc}", tag="den")
                nc.scalar.copy(den[:], avq[HD:HD + 1, :])
                avs = wk.tile([HD, 512], F32, name=f"avs{b}{h}{qc}",
                              tag="avs")
                nc.vector.tensor_copy(avs[:], avq[0:HD, :])
                scr = wk.tile([1, 512], F32, name=f"scr{b}{h}{qc}", tag="scr")
                nc.vector.reciprocal_approx_fast(out=scr[:], in_=den[:])
                recr = wk.tile([1, 512], F32R, name=f"recr{b}{h}{qc}",
                               tag="recr")
                nc.vector._custom_dve(RECIPROCAL_APPROX_NR, out=recr[:],
                                      in0=den[:], in1=scr[:], s0=2.0)
                bc = psA.tile([HD, 512], F32, name=f"bc{b}{h}{qc}",
                              tag="ps_a")
                nc.tensor.matmul(bc[:], ones_r[:], recr[:],
                                 start=True, stop=True)
                attn = wk.tile([HD, 512], BF16, name=f"attn{b}{h}{qc}",
                               tag="attn")
                nc.vector.tensor_mul(attn[:], avs[0:HD, :], bc[:])
                for half in range(2):
                    j = 2 * qc + half
                    eng = nc.sync if half == 0 else nc.gpsimd
                    eng.dma_start(
                        a2a_in[b][128 * j + HD * h:128 * j + HD * (h + 1), :],
                        attn[:, half * 256:(half + 1) * 256])

            def attention(b):
                for h in range(HL):
                    hp = h * 64
                    avp = [psV.tile([HD + 1, 512], F32, name=f"av{b}{h}{q_}",
                                    tag="ps_av") for q_ in range(4)]

                    def s_group(kb, b=b, h=h, hp=hp):
                        """S^T + exp (+mask) for all valid q chunks of kb."""
                        res = []
                        kch = ktc[b * 4 + kb // 4]
                        koff = (kb % 4) * 128
                        for qc in range(kb // 4, 4):
                            # diagonal block: columns [0, 128m) are fully
                            # masked — compute S, exp and the triangular
                            # mask only on the live columns; AV reads the
                            # same restricted slice so the dead region of
                            # pt is never consumed
                            m = kb % 4 if qc == kb // 4 else 0
                            c0 = 128 * m
                            sps = psS.tile([128, 512], F32,
                                           name=f"s{b}{h}{kb}{qc}",
                                           tag="ps_s")
                            nc.tensor.matmul(
                                sps[:, c0:512],
                                kch[hp:hp + 64, koff:koff + 128],
                                qtc[b * 4 + qc][hp:hp + 64, c0:512],
                                start=True, stop=True,
                            )
                            pt = ptp.tile([128, 512], BF16,
                                          name=f"pt{b}{h}{kb}{qc}", tag="pt")
                            nc.scalar.activation(
                                pt[:, c0:512], sps[:, c0:512],
                                mybir.ActivationFunctionType.Exp,
                                scale=0.125,
                            )
                            if qc == kb // 4:
                                nc.vector.tensor_mul(
                                    pt[:, c0:512], pt[:, c0:512],
                                    mask_sb[kb % 4][:, c0:512])
                            res.append((qc, pt, c0))
                        return res

                    def av_group(kb, pts, b=b, h=h, avp=avp):
                        # diagonal chunk last: its pt needs an extra DVE
                        # mask-multiply after the exp
                        pts = pts[1:] + pts[:1] if len(pts) > 1 else pts
                        for qc, pt, c0 in pts:
                            nc.tensor.matmul(
                                avp[qc][:, c0:512],
                                v_sb[b * 16 + kb][:, h * (HD + 1):
                                                  (h + 1) * (HD + 1)],
                                pt[:, c0:512],
                                start=(kb == 0), stop=(kb == 4 * qc + 3),
                                skip_group_check=bool(c0),
                            )

                    # software-pipeline S one kb-group ahead of AV; emit the
                    # normalize chain for q chunk qc right after its AV stop
                    # (kb == 4qc+3) so downstream engines start early.
                    prev = s_group(0)
                    for kb in range(1, 16):
                        cur = s_group(kb)
                        av_group(kb - 1, prev)
                        if (kb - 1) % 4 == 3:
                            qc = (kb - 1) // 4
                            normalize(b, h, qc, avp[qc])
                        prev = cur
                    av_group(15, prev)
                    normalize(b, h, 3, avp[3])

            # ---- phase 5: AllToAll + local full-contraction out-proj ----
            def a2a_issue(b):
                if b == 0:
                    return
                nc.gpsimd.collective_compute(
                    "AllToAll",
                    mybir.AluOpType.bypass,
                    replica_groups=[list(range(NC))],
                    ins=[a2a_in_m[:]],
                    outs=[a2a_out_m[:]],
                )

            def outproj(b):
                att_sb = []
                qs = [nc.gpsimd, nc.sync, nc.scalar]
                for c in range(8):
                    t = pp.tile([128, 256], BF16, name=f"att{b}{c}",
                                tag=f"att{b}{c}")
                    qs[c % 3].dma_start(t[:],
                                        a2a_out[b][c * 128:(c + 1) * 128, :])
                    att_sb.append(t)
                for tb in range(2):
                    for j in range(2):
                        po = psS.tile([128, 512], F32, name=f"po{b}{tb}{j}",
                                      tag="ps_s")
                        for c in range(8):
                            nc.tensor.matmul(
                                po[:],
                                att_sb[c][:, tb * 128:(tb + 1) * 128],
                                wp_sb[c][:, j * 512:(j + 1) * 512],
                                start=(c == 0), stop=(c == 7),
                            )
                        ot = wk.tile([128, 512], F32, name=f"ot{b}{tb}{j}",
                                     tag="ot")
                        eng = nc.vector if (tb + j) % 2 == 0 else nc.scalar
                        if eng is nc.vector:
                            nc.vector.tensor_copy(ot[:], po[:])
                        else:
                            nc.scalar.copy(ot[:], po[:])
                        qs[(tb * 2 + j) % 2].dma_start(
                            out_d[b * 256 + tb * 128:b * 256 + (tb + 1) * 128,
                                  j * 512:(j + 1) * 512],
                            ot[:])

            # ---- main flow ----
            qkv_quarter(0)
            qkv_quarter(1)
            for ch in range(4):       # rope batch 0 (DVE) while PE does th2/3
                rope_chunk(ch)
            qkv_quarter(2)
            qkv_quarter(3)
            for kb in range(16):      # V blocks batch 0
                v_block(kb)
            for ch in range(4, 8):    # rope batch 1 under attention(0)
                rope_chunk(ch)
            for kb in range(16, 32):  # V blocks batch 1
                v_block(kb)

            attention(0)
            a2a_issue(0)
            attention(1)
            a2a_issue(1)
            # outproj(0) fills the PE while a2a(1) is in flight
            outproj(0)
            outproj(1)

    nc.finalize()
    return nc


def host_inputs(x, w_attn, w_proj):
    """Host-side sharding/layout prep. Returns per-core in_maps."""
    x2 = np.ascontiguousarray(x.reshape(BT, C).T).astype(BF)   # [C, BT]

    inv = 1.0 / (ROPE_BASE ** (np.arange(0, HD, 2, dtype=np.float32) / HD))
    tpos = np.arange(T, dtype=np.float32)
    freqs = tpos[:, None] * inv[None, :]                  # [T, 32]
    emb = np.concatenate([freqs, freqs], axis=-1)         # [T, 64]
    cosT = np.cos(emb).T.astype(np.float32)               # [64, T]
    sinT = np.sin(emb).T.astype(np.float32)
    cos_full = np.ascontiguousarray(np.tile(cosT, (2, B))).astype(BF)
    sin_full = np.ascontiguousarray(np.tile(sinT, (2, B))).astype(BF)

    m64 = np.zeros((HD, HD), dtype=np.float32)
    half = HD // 2
    for d in range(half):
        m64[d, d + half] = -1.0
        m64[d + half, d] = 1.0
    perm = np.zeros((128, 128), dtype=np.float32)
    perm[0:HD, 0:HD] = m64
    perm[HD:128, HD:128] = m64
    permT = np.ascontiguousarray(perm.T).astype(BF)

    masks = np.zeros((4, 128, 512), dtype=np.float32)
    qi = np.arange(512)[None, :]
    ki = np.arange(128)[:, None]
    for m in range(4):
        masks[m] = (qi - ki >= m * 128).astype(np.float32)
    masks = masks.astype(BF)

    ident = np.eye(128, dtype=np.float32)
    wpT = np.ascontiguousarray(w_proj.T).astype(BF)       # [c, o]

    in_maps = []
    for i in range(NC):
        r0 = i * (HL * HD)
        wq = w_attn[r0:r0 + HL * HD, :]
        wk_ = w_attn[C + r0:C + r0 + HL * HD, :]
        wv = w_attn[2 * C + r0:2 * C + r0 + HL * HD, :]
        wqkvT = np.ascontiguousarray(
            np.concatenate([wq, wk_, wv], axis=0).T).astype(BF)
        in_maps.append({
            "xT": x2, "wqkvT": wqkvT, "wpT": wpT,
            "cosT": cos_full, "sinT": sin_full, "permT": permT,
            "masks": masks, "ident": ident,
        })
    return in_maps


_NC_CACHE = None


def _get_nc():
    global _NC_CACHE
    if _NC_CACHE is None:
        _NC_CACHE = build()
    return _NC_CACHE


def run(x, w_attn, w_proj, trace=False):
    nc = _get_nc()
    in_maps = host_inputs(np.asarray(x), np.asarray(w_attn),
                          np.asarray(w_proj))
    res = run_bass_kernel_spmd(nc, in_maps, list(range(NC)), trace=trace)
    # core i returns [512, 1024]: rows 0:256 = batch0 rows [256i, 256i+256),
    # rows 256:512 = batch1 rows [256i, 256i+256)
    out = np.empty((B, T, C), dtype=np.float32)
    piece = T // NC
    for i in range(NC):
        sh = res.results[i]["out"]
        out[0, i * piece:(i + 1) * piece] = sh[0:piece]
        out[1, i * piece:(i + 1) * piece] = sh[piece:2 * piece]
    return out, res


def kernel(x, w_attn, w_proj):
    out, _ = run(x, w_attn, w_proj, trace=False)
    return out


# revision 40
# speedup vs baseline: 1.1005x; 1.1005x over previous
"""Causal self-attention with RoPE on 8 TRN2 NeuronCores.

Head-parallel tensor parallelism: core i owns heads 2i, 2i+1. Each core
computes its slice of the qkv projection (bf16 inputs, f32 accumulate),
RoPE via a permutation matmul + DVE muls, then per-head causal
attention entirely in SBUF (bf16 q/k/v, f32 PSUM, exp on the Act
engine with the diagonal blocks column-restricted to the unmasked
range). The softmax-normalized per-head outputs are exchanged with a
per-batch AllToAll (8x less traffic than a ReduceScatter of partial
projections) so every core holds all 1024 attention channels for its
512-row t-shard; the output projection then runs locally over the full
contraction with no cross-core reduction. outproj(batch 0) is emitted
after the batch-1 AllToAll trigger so it fills the collective's wait
window.

PSUM: 1 bank scratch (rope/transpose/den-broadcast), 3 banks for the
S tiles (deep S->exp->AV pipeline) shared with outproj, 4 banks for
the AV accumulators shared with the qkv projection.

Erratum rules respected: no f32r transpose (V transposed in plain f32),
no mixed base partitions inside one PSUM accumulation group, no
bf16/f32r operand mixing in matmuls, no PSUMxPSUM tensor_tensor, no
PSUM or partition-offset inputs to the custom DVE reciprocal ops.
"""

import numpy as np
import ml_dtypes

import concourse.bass as bass
import concourse.mybir as mybir
import concourse.tile as tile
from concourse import bacc
from concourse.bass_utils import run_bass_kernel_spmd
from concourse.dve_ops import RECIPROCAL_APPROX_NR

F32 = mybir.dt.float32
F32R = mybir.dt.float32r
BF16 = mybir.dt.bfloat16

B, T, C = 2, 2048, 1024
H, HD = 16, 64
NC = 8
HL = H // NC          # heads per core = 2
BT = B * T            # 4096
FQKV = 3 * HL * HD    # 384 rows of w_attn per core
TSH = BT // NC        # 512 output rows per core (256 per batch)
NCH = BT // 512       # 8 column chunks of the [*, BT] activations
ROPE_BASE = 10000.0
BF = ml_dtypes.bfloat16


def build():
    nc = bacc.Bacc(None, target_bir_lowering=False)

    xT_d = nc.dram_tensor("xT", [C, BT], BF16, kind="ExternalInput")
    wq_d = nc.dram_tensor("wqkvT", [C, FQKV], BF16, kind="ExternalInput")
    wp_d = nc.dram_tensor("wpT", [C, C], BF16, kind="ExternalInput")
    cos_d = nc.dram_tensor("cosT", [128, BT], BF16, kind="ExternalInput")
    sin_d = nc.dram_tensor("sinT", [128, BT], BF16, kind="ExternalInput")
    perm_d = nc.dram_tensor("permT", [128, 128], BF16, kind="ExternalInput")
    mask_d = nc.dram_tensor("masks", [4, 128, 512], BF16, kind="ExternalInput")
    id_d = nc.dram_tensor("ident", [128, 128], F32, kind="ExternalInput")
    out_d = nc.dram_tensor("out", [TSH, C], F32, kind="ExternalOutput")

    # AllToAll exchange: block j of a2a_in = my 128 attention channels for
    # core j's 256 t-rows of batch b; block j of a2a_out = core j's 128
    # channels for MY 256 t-rows.
    a2a_in = {(b_, h_): nc.dram_tensor(f"a2ain{b_}{h_}", [512, 256], BF16)
              for b_ in range(2) for h_ in range(2)}
    a2a_out = {(b_, h_): nc.dram_tensor(f"a2aout{b_}{h_}", [512, 256], BF16)
               for b_ in range(2) for h_ in range(2)}

    with tile.TileContext(nc) as tc:
        with (
            tc.tile_pool(name="persist", bufs=1) as pp,
            tc.tile_pool(name="work", bufs=2) as wk,
            tc.tile_pool(name="pts", bufs=12) as ptp,
            tc.tile_pool(name="psA", bufs=1, space="PSUM") as psA,
            tc.tile_pool(name="psS", bufs=3, space="PSUM") as psS,
            tc.tile_pool(name="psV", bufs=4, space="PSUM") as psV,
        ):
            # ---- constants / weights (persist) ----
            wq_sb = []
            for c in range(8):
                t = pp.tile([128, FQKV], BF16, name=f"wq{c}", tag=f"wq{c}")
                nc.sync.dma_start(t[:], wq_d[c * 128:(c + 1) * 128, :])
                wq_sb.append(t)
            wp_sb = []
            for c in range(8):
                t = pp.tile([128, C], BF16, name=f"wp{c}", tag=f"wp{c}")
                nc.gpsimd.dma_start(t[:], wp_d[c * 128:(c + 1) * 128, :])
                wp_sb.append(t)
            perm_sb = pp.tile([128, 128], BF16, name="perm_sb", tag="perm_sb")
            nc.gpsimd.dma_start(perm_sb[:], perm_d[:])
            id_sb = pp.tile([128, 128], F32, name="id_sb", tag="id_sb")
            nc.gpsimd.dma_start(id_sb[:], id_d[:])
            mask_sb = []
            for m in range(4):
                t = pp.tile([128, 512], BF16, name=f"mask{m}", tag=f"mask{m}")
                nc.gpsimd.dma_start(t[:], mask_d[m])
                mask_sb.append(t)
            ones_f = pp.tile([128, 1], F32, name="ones_f", tag="ones_f")
            nc.vector.memset(ones_f[:], 1.0)
            ones_c = pp.tile([128, 1], BF16, name="ones_c", tag="ones_c")
            nc.vector.tensor_copy(ones_c[:], ones_f[:])
            ones_r = pp.tile([1, HD], F32R, name="ones_r", tag="ones_r")
            nc.vector.tensor_copy(ones_r[:],
                                  ones_f[0:1, 0:1].broadcast_to((1, HD)))


            # chunked activations: 8 chunks of [128, 512] each
            qtc = [pp.tile([128, 512], BF16, name=f"qtc{i}", tag=f"qtc{i}")
                   for i in range(NCH)]
            ktc = [pp.tile([128, 512], BF16, name=f"ktc{i}", tag=f"ktc{i}")
                   for i in range(NCH)]
            vtc = [pp.tile([128, 512], F32, name=f"vtc{i}", tag=f"vtc{i}")
                   for i in range(NCH)]
            fdst = [qtc, ktc, vtc]
            v_sb = [None] * 32

            # ---- phase 1: qkvT = wqkvT.T @ xT, [f, t] layout ----
            def qkv_quarter(th):
                xt_sb = []
                for c in range(8):
                    t = pp.tile([128, 1024], BF16, name=f"xt{th}{c}",
                                tag=f"xt{c}")
                    nc.sync.dma_start(t[:], xT_d[c * 128:(c + 1) * 128,
                                                 th * 1024:(th + 1) * 1024])
                    xt_sb.append(t)
                for f in range(3):
                    pq = [psV.tile([128, 512], F32, name=f"pq{th}{f}{tq}",
                                   tag="ps_av") for tq in range(2)]
                    for c in range(8):
                        for tq in range(2):
                            nc.tensor.matmul(
                                pq[tq][:],
                                wq_sb[c][:, f * 128:(f + 1) * 128],
                                xt_sb[c][:, tq * 512:(tq + 1) * 512],
                                start=(c == 0), stop=(c == 7),
                            )
                    for tq in range(2):
                        nc.vector.tensor_copy(fdst[f][th * 2 + tq][:],
                                              pq[tq][:])

            # ---- phase 2: RoPE on q, k chunks (in place) ----
            def rope_chunk(ch):
                cosc = wk.tile([128, 512], BF16, name=f"cosc{ch}", tag="cosc")
                nc.gpsimd.dma_start(cosc[:], cos_d[:, ch * 512:(ch + 1) * 512])
                sinc = wk.tile([128, 512], BF16, name=f"sinc{ch}", tag="sinc")
                nc.gpsimd.dma_start(sinc[:], sin_d[:, ch * 512:(ch + 1) * 512])
                for which, tcl in (("q", qtc), ("k", ktc)):
                    src = tcl[ch]
                    pr = psA.tile([128, 512], F32, name=f"pr{which}{ch}",
                                  tag="ps_a")
                    nc.tensor.matmul(pr[:], perm_sb[:], src[:],
                                     start=True, stop=True)
                    rot = wk.tile([128, 512], BF16, name=f"rot{which}{ch}",
                                  tag="rot")
                    nc.vector.tensor_mul(rot[:], pr[:], sinc[:])
                    nc.vector.tensor_mul(src[:], src[:], cosc[:])
                    nc.vector.tensor_add(src[:], src[:], rot[:])

            # ---- phase 3: V blocks [t, d] with ones columns ----
            def v_block(kb):
                pv = psA.tile([128, 128], F32, name=f"pv{kb}", tag="ps_a")
                nc.tensor.transpose(
                    pv[:], vtc[kb // 4][:, (kb % 4) * 128:(kb % 4 + 1) * 128],
                    id_sb[:])
                v = pp.tile([128, 2 * (HD + 1)], BF16, name=f"v{kb}",
                            tag=f"v{kb}")
                nc.vector.tensor_copy(v[:, 0:HD], pv[:, 0:HD])
                nc.vector.tensor_copy(v[:, HD + 1:2 * HD + 1],
                                      pv[:, HD:2 * HD])
                nc.vector.tensor_copy(v[:, HD:HD + 1], ones_c[:])
                nc.vector.tensor_copy(v[:, 2 * HD + 1:2 * HD + 2], ones_c[:])
                v_sb[kb] = v

            # ---- phase 4: attention per (batch, head), kb-outer ----
            def normalize(b, h, qc, avq):
                """softmax-normalize avq rows 0:64 by the ones-row 64 and
                stage the result into the AllToAll input buffer."""
                den = wk.tile([1, 512], F32, name=f"den{b}{h}{qc}", tag="den")
                nc.scalar.copy(den[:], avq[HD:HD + 1, :])
                avs = wk.tile([HD, 512], F32, name=f"avs{b}{h}{qc}",
                              tag="avs")
                nc.vector.tensor_copy(avs[:], avq[0:HD, :])
                scr = wk.tile([1, 512], F32, name=f"scr{b}{h}{qc}", tag="scr")
                nc.vector.reciprocal_approx_fast(out=scr[:], in_=den[:])
                recr = wk.tile([1, 512], F32R, name=f"recr{b}{h}{qc}",
                               tag="recr")
                nc.vector._custom_dve(RECIPROCAL_APPROX_NR, out=recr[:],
                                      in0=den[:], in1=scr[:], s0=2.0)
                bc = psA.tile([HD, 512], F32, name=f"bc{b}{h}{qc}",
                              tag="ps_a")
                nc.tensor.matmul(bc[:], ones_r[:], recr[:],
                                 start=True, stop=True)
                attn = wk.tile([HD, 512], BF16, name=f"attn{b}{h}{qc}",
                               tag="attn")
                nc.vector.tensor_mul(attn[:], avs[0:HD, :], bc[:])
                for half in range(2):
                    j = 2 * qc + half
                    eng = nc.sync if half == 0 else nc.gpsimd
                    eng.dma_start(
                        a2a_in[b, h][HD * j:HD * (j + 1), :],
                        attn[:, half * 256:(half + 1) * 256])

            def attention(b):
                for h in range(HL):
                    hp = h * 64
                    avp = [psV.tile([HD + 1, 512], F32, name=f"av{b}{h}{q_}",
                                    tag="ps_av") for q_ in range(4)]

                    def s_group(kb, b=b, h=h, hp=hp):
                        """S^T + exp (+mask) for all valid q chunks of kb."""
                        res = []
                        kch = ktc[b * 4 + kb // 4]
                        koff = (kb % 4) * 128
                        for qc in range(kb // 4, 4):
                            # diagonal block: columns [0, 128m) are fully
                            # masked — compute S, exp and the triangular
                            # mask only on the live columns; AV reads the
                            # same restricted slice so the dead region of
                            # pt is never consumed
                            m = kb % 4 if qc == kb // 4 else 0
                            c0 = 128 * m
                            sps = psS.tile([128, 512], F32,
                                           name=f"s{b}{h}{kb}{qc}",
                                           tag="ps_s")
                            nc.tensor.matmul(
                                sps[:, c0:512],
                                kch[hp:hp + 64, koff:koff + 128],
                                qtc[b * 4 + qc][hp:hp + 64, c0:512],
                                start=True, stop=True,
                            )
                            pt = ptp.tile([128, 512], BF16,
                                          name=f"pt{b}{h}{kb}{qc}", tag="pt")
                            nc.scalar.activation(
                                pt[:, c0:512], sps[:, c0:512],
                                mybir.ActivationFunctionType.Exp,
                                scale=0.125,
                            )
                            if qc == kb // 4:
                                nc.vector.tensor_mul(
                                    pt[:, c0:512], pt[:, c0:512],
                                    mask_sb[kb % 4][:, c0:512])
                            res.append((qc, pt, c0))
                        return res

                    def av_group(kb, pts, b=b, h=h, avp=avp):
                        # diagonal chunk last: its pt needs an extra DVE
                        # mask-multiply after the exp
                        pts = pts[1:] + pts[:1] if len(pts) > 1 else pts
                        for qc, pt, c0 in pts:
                            nc.tensor.matmul(
                                avp[qc][:, c0:512],
                                v_sb[b * 16 + kb][:, h * (HD + 1):
                                                  (h + 1) * (HD + 1)],
                                pt[:, c0:512],
                                start=(kb == 0), stop=(kb == 4 * qc + 3),
                                skip_group_check=bool(c0),
                            )

                    # software-pipeline S one kb-group ahead of AV; emit the
                    # normalize chain for q chunk qc right after its AV stop
                    # (kb == 4qc+3) so downstream engines start early.
                    prev = s_group(0)
                    for kb in range(1, 16):
                        cur = s_group(kb)
                        av_group(kb - 1, prev)
                        if (kb - 1) % 4 == 3:
                            qc = (kb - 1) // 4
                            normalize(b, h, qc, avp[qc])
                        prev = cur
                    av_group(15, prev)
                    normalize(b, h, 3, avp[3])
                    a2a_issue(b, h)

            # ---- phase 5: AllToAll + local full-contraction out-proj ----
            def a2a_issue(b, h):
                nc.gpsimd.collective_compute(
                    "AllToAll",
                    mybir.AluOpType.bypass,
                    replica_groups=[list(range(NC))],
                    ins=[a2a_in[b, h][:]],
                    outs=[a2a_out[b, h][:]],
                )

            def outproj(b):
                att_sb = []
                qs = [nc.gpsimd, nc.sync, nc.scalar]
                for c in range(8):
                    t = pp.tile([128, 256], BF16, name=f"att{b}{c}",
                                tag=f"att{b}{c}")
                    for h in range(2):
                        qs[(2 * c + h) % 3].dma_start(
                            t[HD * h:HD * (h + 1), :],
                            a2a_out[b, h][c * HD:(c + 1) * HD, :])
                    att_sb.append(t)
                for tb in range(2):
                    for j in range(2):
                        po = psS.tile([128, 512], F32, name=f"po{b}{tb}{j}",
                                      tag="ps_s")
                        for c in range(8):
                            nc.tensor.matmul(
                                po[:],
                                att_sb[c][:, tb * 128:(tb + 1) * 128],
                                wp_sb[c][:, j * 512:(j + 1) * 512],
                                start=(c == 0), stop=(c == 7),
                            )
                        ot = wk.tile([128, 512], F32, name=f"ot{b}{tb}{j}",
                                     tag="ot")
                        eng = nc.vector if (tb + j) % 2 == 0 else nc.scalar
                        if eng is nc.vector:
                            nc.vector.tensor_copy(ot[:], po[:])
                        else:
                            nc.scalar.copy(ot[:], po[:])
                        qs[(tb * 2 + j) % 2].dma_start(
                            out_d[b * 256 + tb * 128:b * 256 + (tb + 1) * 128,
                                  j * 512:(j + 1) * 512],
                            ot[:])

            # ---- main flow ----
            qkv_quarter(0)
            qkv_quarter(1)
            for ch in range(4):       # rope batch 0 (DVE) while PE does th2/3
                rope_chunk(ch)
            qkv_quarter(2)
            qkv_quarter(3)
            for kb in range(16):      # V blocks batch 0
                v_block(kb)
            for ch in range(4, 8):    # rope batch 1 under attention(0)
                rope_chunk(ch)
            for kb in range(16, 32):  # V blocks batch 1
                v_block(kb)

            attention(0)
            attention(1)
            # outproj(0) fills the PE while a2a(1,h1) is in flight
            outproj(0)
            outproj(1)

    nc.finalize()
    return nc


def host_inputs(x, w_attn, w_proj):
    """Host-side sharding/layout prep. Returns per-core in_maps."""
    x2 = np.ascontiguousarray(x.reshape(BT, C).T).astype(BF)   # [C, BT]

    inv = 1.0 / (ROPE_BASE ** (np.arange(0, HD, 2, dtype=np.float32) / HD))
    tpos = np.arange(T, dtype=np.float32)
    freqs = tpos[:, None] * inv[None, :]                  # [T, 32]
    emb = np.concatenate([freqs, freqs], axis=-1)         # [T, 64]
    cosT = np.cos(emb).T.astype(np.float32)               # [64, T]
    sinT = np.sin(emb).T.astype(np.float32)
    cos_full = np.ascontiguousarray(np.tile(cosT, (2, B))).astype(BF)
    sin_full = np.ascontiguousarray(np.tile(sinT, (2, B))).astype(BF)

    m64 = np.zeros((HD, HD), dtype=np.float32)
    half = HD // 2
    for d in range(half):
        m64[d, d + half] = -1.0
        m64[d + half, d] = 1.0
    perm = np.zeros((128, 128), dtype=np.float32)
    perm[0:HD, 0:HD] = m64
    perm[HD:128, HD:128] = m64
    permT = np.ascontiguousarray(perm.T).astype(BF)

    masks = np.zeros((4, 128, 512), dtype=np.float32)
    qi = np.arange(512)[None, :]
    ki = np.arange(128)[:, None]
    for m in range(4):
        masks[m] = (qi - ki >= m * 128).astype(np.float32)
    masks = masks.astype(BF)

    ident = np.eye(128, dtype=np.float32)
    wpT = np.ascontiguousarray(w_proj.T).astype(BF)       # [c, o]

    in_maps = []
    for i in range(NC):
        r0 = i * (HL * HD)
        wq = w_attn[r0:r0 + HL * HD, :]
        wk_ = w_attn[C + r0:C + r0 + HL * HD, :]
        wv = w_attn[2 * C + r0:2 * C + r0 + HL * HD, :]
        wqkvT = np.ascontiguousarray(
            np.concatenate([wq, wk_, wv], axis=0).T).astype(BF)
        in_maps.append({
            "xT": x2, "wqkvT": wqkvT, "wpT": wpT,
            "cosT": cos_full, "sinT": sin_full, "permT": permT,
            "masks": masks, "ident": ident,
        })
    return in_maps


_NC_CACHE = None


def _get_nc():
    global _NC_CACHE
    if _NC_CACHE is None:
        _NC_CACHE = build()
    return _NC_CACHE


def run(x, w_attn, w_proj, trace=False):
    nc = _get_nc()
    in_maps = host_inputs(np.asarray(x), np.asarray(w_attn),
                          np.asarray(w_proj))
    res = run_bass_kernel_spmd(nc, in_maps, list(range(NC)), trace=trace)
    # core i returns [512, 1024]: rows 0:256 = batch0 rows [256i, 256i+256),
    # rows 256:512 = batch1 rows [256i, 256i+256)
    out = np.empty((B, T, C), dtype=np.float32)
    piece = T // NC
    for i in range(NC):
        sh = res.results[i]["out"]
        out[0, i * piece:(i + 1) * piece] = sh[0:piece]
        out[1, i * piece:(i + 1) * piece] = sh[piece:2 * piece]
    return out, res


def kernel(x, w_attn, w_proj):
    out, _ = run(x, w_attn, w_proj, trace=False)
    return out
